# revision 10
# baseline (speedup 1.0000x reference)
"""ColBERT intra-batch MaxSim scoring kernel for 8 Trainium2 NeuronCores.

Math (see reference):
  Q = l2norm(q_hidden @ W.T)                       [B, LQ, DIM]
  D = l2norm(d_hidden @ W.T); D masked             [B, LD, DIM]
  sim[b,c,q,k] = Q[b,q]·D[c,k]; masked k -> -inf
  out[b,c] = sum_q max_k sim

Sharding: docs (dim c) are sharded 16-per-core; q_hidden/W replicated.
Each core computes its [B, 16] slice of the score matrix.

Device-side structure:
  * Host pre-transposes activations to [HID, tokens] and converts to bf16
    (halves HBM traffic; verified 9.6e-4 rel err vs 2e-2 budget).
  * The doc mask is folded away on the host: each doc's valid tokens are
    gathered to the front and the tail is padded with copies of the doc's
    first valid token, so the device kernel needs no masking.
  * All input DMAs are issued on the sync queue in priority order (wt,
    dT chunks, then qT column groups).  One HWDGE ring = strict FIFO, so
    dT gets full bandwidth first and the doc pipeline starts ~15us in;
    qT groups trickle in behind at the rate the sim tiles consume them.
  * Q is NOT normalized before the sim matmul: max_k is invariant under a
    positive per-query scale, so 1/|Q| is folded into the block-ones
    lhsT of the final query-sum matmul.
  * D norms: ones-matmul gives sumsq as a [1, NVT] row; sqrt (ACT) +
    2-ULP approximate reciprocal (DVE) give 1/|D| which is broadcast to
    128 partitions with a K=1 ones outer-product matmul and multiplied in.
  * ACT spline tables for Square/Sqrt are warmed at t=0 so the first
    normalize step doesn't eat the ~2.6us table-load latency.
"""

import os

import numpy as np

B, LQ, LD, HID, DIM = 128, 32, 256, 768, 128
NCORES = 8
DPC = B // NCORES          # docs per core
TQ = B * LQ                # total query tokens
KC = HID // 128            # contraction chunks for the projection

SIM_MODE = os.environ.get("KERNEL_SIM_MODE", "bf16")
# fold path: tiles with t % FOLD_SKIP != FOLD_SKIP-1 take the ACT-convert +
# DVE TT-max-fold route (cheaper on DVE, costs ACT); 0 disables folding
FOLD_SKIP = int(os.environ.get("KERNEL_FOLD_SKIP", "3"))

# qT column groups, in DMA priority order (first groups smaller so the
# first sim tiles can start as early as possible)
QGROUPS = [(0, 512), (512, 512), (1024, 1024), (2048, 1024), (3072, 1024)]


def _chunks(total, step):
    """[(off, len)] cut at `step` boundaries — a matmul's PSUM output must
    stay inside a single 512-float bank, so chunks may never straddle one."""
    return [(o, min(step, total - o)) for o in range(0, total, step)]


def _qgroup_of(j):
    """(group index, column offset within group) for 512-col chunk j."""
    off = j * 512
    for gi, (go, gw) in enumerate(QGROUPS):
        if go <= off < go + gw:
            return gi, off - go
    raise ValueError(j)


def _build_program(NV):
    import concourse.bass as bass  # noqa: F401
    import concourse.tile as tile
    from concourse import bacc, mybir

    f32 = mybir.dt.float32
    bf16 = mybir.dt.bfloat16
    AF = mybir.ActivationFunctionType
    AX = mybir.AxisListType
    ALU = mybir.AluOpType

    proj_dt = bf16
    sim_dt = {"bf16": bf16, "f32": f32}[SIM_MODE]
    sq_dt = bf16

    NVT = DPC * NV          # compacted doc tokens per core
    NVH = NVT // 2          # half (8 docs) — one PSUM sim tile
    NQCH = TQ // 512        # q-projection column chunks
    NTT = TQ // 128         # sim lhsT tiles (query-token tiles)
    BPT = 128 // LQ         # batch entries per query-token tile
    d_chunks = _chunks(NVT, 512)   # d-projection column chunks
    s_chunks = _chunks(NVH, 512)   # sim matmul N chunks per half

    nc = bacc.Bacc(
        "TRN2",
        target_bir_lowering=False,
        debug=False,
        num_devices=NCORES,
    )

    qT_d = nc.dram_tensor("qT", [HID, TQ], proj_dt, kind="ExternalInput")
    dT_d = nc.dram_tensor("dT", [HID, NVT], proj_dt, kind="ExternalInput")
    wT_d = nc.dram_tensor("wT", [128, KC, DIM], proj_dt, kind="ExternalInput")
    qso_d = nc.dram_tensor("qso", [128, BPT], f32, kind="ExternalInput")
    onescol_d = nc.dram_tensor("onescol", [128, 1], sq_dt, kind="ExternalInput")
    onesrow_d = nc.dram_tensor("onesrow", [1, 128], f32, kind="ExternalInput")
    out_d = nc.dram_tensor("out", [B, DPC], f32, kind="ExternalOutput")

    # [HID, t] rows seen as (k, p): row = k*128 + p
    qT_v = qT_d[:, :].rearrange("(k p) t -> p k t", p=128)

    with tile.TileContext(nc) as tc, tc.tile_pool(name="persist", bufs=1) as per:
        # --- constants + persistent SBUF tensors ---------------------------
        wt = per.tile([128, KC, DIM], proj_dt, name="wt")
        qso = per.tile([128, BPT], f32, name="qso")
        onescol = per.tile([128, 1], sq_dt, name="onescol")
        onescol8 = per.tile([128, 8], sq_dt, name="onescol8")
        onesrow = per.tile([1, 128], f32, name="onesrow")
        warm = per.tile([1, 16], f32, name="warm")
        QT = per.tile([128, TQ], sim_dt, name="QT")       # q-proj [d, t] unnormalized
        DTn = per.tile([128, NVT], sim_dt, name="DTn")    # normalized d-proj
        invnQ = per.tile([128, NTT], f32, name="invnQ")   # 1/|Q| per query token
        normQ = per.tile([128, NTT], f32, name="normQ")
        lhsQ = per.tile([128, NTT, BPT], f32, name="lhsQ")  # blockones * 1/|Q|
        invnD_row = per.tile([1, NVT], f32, name="invnD_row")
        rowtmp = per.tile([1, NVT], f32, name="rowtmp")
        outstage = per.tile([BPT, NTT * DPC], f32, name="outstage")
        dts = [per.tile([128, NVT], proj_dt, name=f"dt{k}") for k in range(KC)]
        qtg = [
            per.tile([128, KC, gw], proj_dt, name=f"qtg{gi}")
            for gi, (_, gw) in enumerate(QGROUPS)
        ]

        # input DMAs: one ring (sync), strict priority order
        nc.sync.dma_start(wt[:], wT_d[:, :, :])
        for k in range(KC):
            nc.sync.dma_start(dts[k][:], dT_d[k * 128:(k + 1) * 128, :])
        for gi, (go, gw) in enumerate(QGROUPS):
            nc.sync.dma_start(qtg[gi][:, :, :], qT_v[:, :, go:go + gw])
        # tiny constants off the critical ring (SWDGE)
        nc.gpsimd.dma_start(qso[:], qso_d[:, :])
        nc.gpsimd.dma_start(onescol[:], onescol_d[:, :])
        nc.gpsimd.dma_start(onesrow[:], onesrow_d[:, :])

        # warm the ACT spline tables while DMAs are in flight
        nc.vector.memset(warm[:], 1.0)
        nc.vector.memset(onescol8[:], 1.0)
        nc.scalar.activation(warm[:], warm[:], AF.Square)
        nc.scalar.activation(warm[:], warm[:], AF.Sqrt)

        # ---------------- phase D: project doc tokens ----------------------
        # k-outer accumulation into one wide PSUM tensor so compute starts
        # as soon as the first dT k-chunk lands.
        with (
            tc.tile_pool(name="psD", bufs=1, space="PSUM") as psD,
            tc.tile_pool(name="ssD", bufs=1, space="PSUM") as ssD,
            tc.tile_pool(name="sqD_pool", bufs=2) as sqD_pool,
            tc.tile_pool(name="bc_pool", bufs=2) as bc_pool,
            tc.tile_pool(name="psB", bufs=2, space="PSUM") as psB,
        ):
            psd = psD.tile([128, NVT], f32, name="psd")
            for k in range(KC):
                for (off, ln) in d_chunks:
                    nc.tensor.matmul(
                        psd[:, off:off + ln],
                        wt[:, k, :],
                        dts[k][:, off:off + ln],
                        start=(k == 0),
                        stop=(k == KC - 1),
                    )
            # chunk-granular norm chain: Square (ACT) -> M=8 ones matmul (PE,
            # full-rate) -> sqrt row straight from PSUM (ACT) -> ~51-ULP
            # reciprocal (DVE); stages pipeline across the 5 chunks
            for (off, ln) in d_chunks:
                sl = slice(off, off + ln)
                sq = sqD_pool.tile([128, 512], sq_dt, name="sqd", tag="sq")
                nc.scalar.activation(sq[:, :ln], psd[:, sl], AF.Square)
                ssd = ssD.tile([8, 512], f32, name="ssd", tag="ssd")
                nc.tensor.matmul(
                    ssd[:, :ln], onescol8[:], sq[:, :ln], start=True, stop=True
                )
                nc.scalar.activation(rowtmp[0:1, sl], ssd[0:1, :ln], AF.Sqrt)
                nc.vector.reciprocal_approx_fast(
                    invnD_row[0:1, sl], rowtmp[0:1, sl]
                )

            # broadcast 1/|D| across partitions and scale D straight out of
            # the projection PSUM (psd stays live until here)
            for (off, ln) in d_chunks:
                sl = slice(off, off + ln)
                psb = psB.tile([128, 512], f32, name="psb", tag="psb")
                nc.tensor.matmul(
                    psb[:, :ln], onesrow[:], invnD_row[:, sl], start=True, stop=True
                )
                bc = bc_pool.tile([128, 512], f32, name="bcast_sb", tag="bc")
                nc.scalar.copy(bc[:, :ln], psb[:, :ln])
                nc.vector.tensor_tensor(
                    DTn[:, sl], psd[:, sl], bc[:, :ln], op=ALU.mult
                )

        # ---------- phase Q+S: project query chunks, sim tiles interleaved --
        # Q-projection chunk j feeds sim tiles t=4j..4j+3; chunks are traced
        # two groups ahead of their sim tiles so the PE never starves the DVE
        # reduce pipeline.  pssim is a single 5-bank tensor whose two halves
        # ping-pong between PE writes and DVE reduces.
        with (
            tc.tile_pool(name="psQS", bufs=2, space="PSUM") as psQS,
            tc.tile_pool(name="ssQ", bufs=1, space="PSUM") as ssQ,
            tc.tile_pool(name="sqQ_pool", bufs=2) as sqQ_pool,
            tc.tile_pool(name="psO", bufs=1, space="PSUM") as psO,
            tc.tile_pool(name="m_pool", bufs=2) as m_pool,
            tc.tile_pool(name="fold_pool", bufs=2) as fold_pool,
        ):
            ssq = ssQ.tile([128, NTT], f32, name="ssq")
            psout = psO.tile([BPT, NTT * DPC], f32, name="psout")

            psq_live = {}

            def project_mm(j, ks):
                gi, r = _qgroup_of(j)
                if j not in psq_live:
                    psq_live[j] = psQS.tile([128, NVH], f32, name="psq", tag="big")
                psq = psq_live[j]
                for k in ks:
                    nc.tensor.matmul(
                        psq[:, 0:512],
                        wt[:, k, :],
                        qtg[gi][:, k, r:r + 512],
                        start=(k == 0),
                        stop=(k == KC - 1),
                    )

            def project(j):
                sl = slice(j * 512, (j + 1) * 512)
                psq = psq_live.pop(j)
                nc.scalar.copy(QT[:, sl], psq[:, 0:512])
                sq = sqQ_pool.tile([128, 512], sq_dt, name="sqq", tag="sqq")
                nc.scalar.activation(sq[:], psq[:, 0:512], AF.Square)
                for s in range(4):
                    col = j * 4 + s
                    nc.tensor.matmul(
                        ssq[:, col:col + 1],
                        sq[:, s * 128:(s + 1) * 128],
                        onescol[:],
                        start=True,
                        stop=True,
                    )
                # per-chunk 1/|Q| and the weighted block-ones lhsT
                csl = slice(j * 4, (j + 1) * 4)
                nc.scalar.activation(normQ[:, csl], ssq[:, csl], AF.Sqrt)
                nc.vector.reciprocal(invnQ[:, csl], normQ[:, csl])
                nc.vector.tensor_tensor(
                    lhsQ[:, csl, :],
                    qso[:].unsqueeze(1).broadcast_to((128, 4, BPT)),
                    invnQ[:, csl].unsqueeze(2).broadcast_to((128, 4, BPT)),
                    op=ALU.mult,
                )

            def simtile(t, weave=None):
                lq = QT[:, t * 128:(t + 1) * 128]
                mall = m_pool.tile([128, DPC], f32, name="mall", tag="mall")
                fold = FOLD_SKIP > 0 and (t % FOLD_SKIP) != FOLD_SKIP - 1
                if fold:
                    sbt = fold_pool.tile([128, NVT], sim_dt, name="sbt", tag="sbt")
                for h in range(2):
                    base = h * NVH
                    ps = psQS.tile([128, NVH], f32, name="pssim", tag="big")
                    for (off, ln) in s_chunks:
                        nc.tensor.matmul(
                            ps[:, off:off + ln],
                            lq,
                            DTn[:, base + off:base + off + ln],
                            start=True,
                            stop=True,
                        )
                    if fold:
                        # evict to SBUF bf16 on ACT; DVE folds below
                        nc.scalar.copy(sbt[:, base:base + NVH], ps[:])
                    else:
                        nc.vector.reduce_max(
                            mall[:, h * (DPC // 2):(h + 1) * (DPC // 2)],
                            ps[:].rearrange("p (g v) -> p g v", v=NV),
                            axis=AX.X,
                        )
                    if weave:
                        weave(h)
                if fold:
                    # two 2x-rate TT-max fold levels, then a short 1x reduce
                    v2, v4 = NV // 2, NV // 4
                    l1 = fold_pool.tile([128, NVT // 2], sim_dt, name="l1", tag="l1")
                    sv = sbt[:].rearrange("p (g v) -> p g v", v=NV)
                    nc.vector.tensor_tensor(
                        l1[:].rearrange("p (g v) -> p g v", v=v2),
                        sv[:, :, 0:v2], sv[:, :, v2:NV], op=ALU.max,
                    )
                    l2 = fold_pool.tile([128, NVT // 4], sim_dt, name="l2", tag="l2")
                    lv = l1[:].rearrange("p (g v) -> p g v", v=v2)
                    nc.vector.tensor_tensor(
                        l2[:].rearrange("p (g v) -> p g v", v=v4),
                        lv[:, :, 0:v4], lv[:, :, v4:v2], op=ALU.max,
                    )
                    nc.vector.reduce_max(
                        mall[:],
                        l2[:].rearrange("p (g v) -> p g v", v=v4),
                        axis=AX.X,
                    )
                nc.tensor.matmul(
                    psout[:, t * DPC:(t + 1) * DPC],
                    lhsQ[:, t, :],
                    mall[:],
                    start=True,
                    stop=True,
                )

            project_mm(0, range(KC))
            project(0)
            project_mm(1, range(KC))
            project(1)
            for j in range(NQCH):
                # weave next-next chunk's six projection matmuls one at a
                # time between sim halves so the DVE reduce never starves
                kstep = iter(range(KC))

                def weave(h, _j=j, _ks=kstep):
                    if _j + 2 < NQCH:
                        k = next(_ks, None)
                        if k is not None:
                            project_mm(_j + 2, [k])

                for ti, t in enumerate(range(j * 4, (j + 1) * 4)):
                    simtile(t, weave=weave)
                    if j + 2 < NQCH and ti == 2:
                        project(j + 2)
            nc.scalar.copy(outstage[:], psout[:])
            nc.sync.dma_start(
                out_d[:, :].rearrange("(t f) c -> f t c", f=BPT),
                outstage[:].rearrange("f (t c) -> f t c", c=DPC),
            )

    nc.compile()
    return nc


def _host_prep(q_hidden, d_hidden, W, d_mask):
    import ml_dtypes

    bf = ml_dtypes.bfloat16
    q = np.ascontiguousarray(np.asarray(q_hidden, dtype=np.float32))
    d = np.ascontiguousarray(np.asarray(d_hidden, dtype=np.float32))
    w = np.ascontiguousarray(np.asarray(W, dtype=np.float32))
    mask = np.asarray(d_mask, dtype=bool)

    nv = mask.sum(axis=1)
    NV = int(-(-max(int(nv.max()), 16) // 8) * 8)
    NV = min(NV, ((LD + 7) // 8) * 8)

    # per-doc gather indices: valid tokens first, padded with the first
    # valid token (duplicates never change a max)
    idx = np.zeros((B, NV), dtype=np.intp)
    for c in range(B):
        v = np.flatnonzero(mask[c])
        row = np.full(NV, v[0], dtype=np.intp)
        row[:min(len(v), NV)] = v[:NV]
        idx[c] = row

    dG = d[np.arange(B)[:, None], idx, :]          # [B, NV, HID]

    qT = np.ascontiguousarray(q.reshape(TQ, HID).T.astype(bf))   # [HID, TQ]
    # W.T rearranged so the [128, KC, DIM] SBUF tile is one contiguous DMA:
    # wTp[p, k, d] = W[d, k*128+p]
    wT = np.ascontiguousarray(
        w.T.reshape(KC, 128, DIM).transpose(1, 0, 2).astype(bf)
    )
    dT_cores = []
    for m in range(NCORES):
        blk = dG[m * DPC:(m + 1) * DPC].reshape(DPC * NV, HID)
        dT_cores.append(np.ascontiguousarray(blk.T.astype(bf)))  # [HID, DPC*NV]

    qso = np.zeros((128, 128 // LQ), dtype=np.float32)
    for p in range(128):
        qso[p, p // LQ] = 1.0
    onescol = np.ones((128, 1), dtype=bf)
    onesrow = np.ones((1, 128), dtype=np.float32)
    return NV, qT, wT, dT_cores, qso, onescol, onesrow


def kernel(q_hidden, d_hidden, W, d_mask):
    from concourse.bass_utils import run_bass_kernel_spmd

    NV, qT, wT, dT_cores, qso, onescol, onesrow = _host_prep(
        q_hidden, d_hidden, W, d_mask
    )
    nc = _build_program(NV)

    in_maps = [
        {
            "qT": qT,
            "dT": dT_cores[m],
            "wT": wT,
            "qso": qso,
            "onescol": onescol,
            "onesrow": onesrow,
        }
        for m in range(NCORES)
    ]
    res = run_bass_kernel_spmd(nc, in_maps, core_ids=list(range(NCORES)))
    out = np.concatenate(
        [res.results[m]["out"] for m in range(NCORES)], axis=1
    )
    return np.ascontiguousarray(out.astype(np.float32))


# revision 16
# speedup vs baseline: 1.1381x; 1.1381x over previous
"""ColBERT intra-batch MaxSim scoring kernel for 8 Trainium2 NeuronCores.

Math (see reference):
  Q = l2norm(q_hidden @ W.T)                       [B, LQ, DIM]
  D = l2norm(d_hidden @ W.T); D masked             [B, LD, DIM]
  sim[b,c,q,k] = Q[b,q]·D[c,k]; masked k -> -inf
  out[b,c] = sum_q max_k sim

Sharding: docs (dim c) are sharded 16-per-core; q_hidden/W replicated.
Each core computes its [B, 16] slice of the score matrix.

Device-side structure:
  * Host pre-transposes activations to [HID, tokens] and converts to bf16
    (halves HBM traffic; verified 9.6e-4 rel err vs 2e-2 budget).
  * The doc mask is folded away on the host: each doc's valid tokens are
    gathered to the front and the tail is padded with copies of the doc's
    first valid token, so the device kernel needs no masking.
  * All input DMAs are issued on the sync queue in priority order (wt,
    dT chunks, then qT column groups).  One HWDGE ring = strict FIFO, so
    dT gets full bandwidth first and the doc pipeline starts ~15us in;
    qT groups trickle in behind at the rate the sim tiles consume them.
  * Q is NOT normalized before the sim matmul: max_k is invariant under a
    positive per-query scale, so 1/|Q| is folded into the block-ones
    lhsT of the final query-sum matmul.
  * D norms: ones-matmul gives sumsq as a [1, NVT] row; sqrt (ACT) +
    2-ULP approximate reciprocal (DVE) give 1/|D| which is broadcast to
    128 partitions with a K=1 ones outer-product matmul and multiplied in.
  * ACT spline tables for Square/Sqrt are warmed at t=0 so the first
    normalize step doesn't eat the ~2.6us table-load latency.
"""

import os

import numpy as np

B, LQ, LD, HID, DIM = 128, 32, 256, 768, 128
NCORES = 8
DPC = B // NCORES          # docs per core
TQ = B * LQ                # total query tokens
KC = HID // 128            # contraction chunks for the projection

SIM_MODE = os.environ.get("KERNEL_SIM_MODE", "bf16")
# fold path: most tiles take the ACT-convert + DVE TT-max-fold route
# (cheaper on DVE, costs ACT).  The first two tiles and every 4th tile
# from t=6 stay on the direct DVE-reduce path so the DVE starts without
# waiting for ACT, and the two engines stay load-balanced (9 direct / 23
# fold ≈ 66us each at measured per-tile costs).
FOLD = os.environ.get("KERNEL_FOLD", "1") == "1"


def _is_direct(t):
    if not FOLD:
        return True
    return t < 2 or (t >= 6 and (t - 6) % 4 == 0)

# qT column groups, in DMA priority order (first groups smaller so the
# first sim tiles can start as early as possible)
QGROUPS = [(0, 512), (512, 512), (1024, 1024), (2048, 1024), (3072, 1024)]


def _chunks(total, step):
    """[(off, len)] cut at `step` boundaries — a matmul's PSUM output must
    stay inside a single 512-float bank, so chunks may never straddle one."""
    return [(o, min(step, total - o)) for o in range(0, total, step)]


def _qgroup_of(j):
    """(group index, column offset within group) for 512-col chunk j."""
    off = j * 512
    for gi, (go, gw) in enumerate(QGROUPS):
        if go <= off < go + gw:
            return gi, off - go
    raise ValueError(j)


def _build_program(NV):
    import concourse.bass as bass  # noqa: F401
    import concourse.tile as tile
    from concourse import bacc, mybir

    f32 = mybir.dt.float32
    bf16 = mybir.dt.bfloat16
    AF = mybir.ActivationFunctionType
    AX = mybir.AxisListType
    ALU = mybir.AluOpType

    proj_dt = bf16
    sim_dt = {"bf16": bf16, "f32": f32}[SIM_MODE]
    sq_dt = bf16

    NVT = DPC * NV          # compacted doc tokens per core
    NVH = NVT // 2          # half (8 docs) — one PSUM sim tile
    NQCH = TQ // 512        # q-projection column chunks
    NTT = TQ // 128         # sim lhsT tiles (query-token tiles)
    BPT = 128 // LQ         # batch entries per query-token tile
    d_chunks = _chunks(NVT, 512)   # d-projection column chunks
    s_chunks = _chunks(NVH, 512)   # sim matmul N chunks per half

    nc = bacc.Bacc(
        "TRN2",
        target_bir_lowering=False,
        debug=False,
        num_devices=NCORES,
    )

    qT_d = nc.dram_tensor("qT", [HID, TQ], proj_dt, kind="ExternalInput")
    dT_d = nc.dram_tensor("dT", [HID, NVT], proj_dt, kind="ExternalInput")
    wT_d = nc.dram_tensor("wT", [128, KC, DIM], proj_dt, kind="ExternalInput")
    qso_d = nc.dram_tensor("qso", [128, BPT], f32, kind="ExternalInput")
    onescol_d = nc.dram_tensor("onescol", [128, 1], sq_dt, kind="ExternalInput")
    onesrow_d = nc.dram_tensor("onesrow", [1, 128], f32, kind="ExternalInput")
    out_d = nc.dram_tensor("out", [B, DPC], f32, kind="ExternalOutput")

    # [HID, t] rows seen as (k, p): row = k*128 + p
    qT_v = qT_d[:, :].rearrange("(k p) t -> p k t", p=128)

    with tile.TileContext(nc) as tc, tc.tile_pool(name="persist", bufs=1) as per:
        # --- constants + persistent SBUF tensors ---------------------------
        wt = per.tile([128, KC, DIM], proj_dt, name="wt")
        qso = per.tile([128, BPT], f32, name="qso")
        onescol = per.tile([128, 1], sq_dt, name="onescol")
        onescol8 = per.tile([128, 8], sq_dt, name="onescol8")
        onesrow = per.tile([1, 128], f32, name="onesrow")
        warm = per.tile([1, 16], f32, name="warm")
        QT = per.tile([128, TQ], sim_dt, name="QT")       # q-proj [d, t] unnormalized
        DTn = per.tile([128, NVT], sim_dt, name="DTn")    # normalized d-proj
        invnQ = per.tile([128, NTT], f32, name="invnQ")   # 1/|Q| per query token
        normQ = per.tile([128, NTT], f32, name="normQ")
        lhsQ = per.tile([128, NTT, BPT], f32, name="lhsQ")  # blockones * 1/|Q|
        invnD_row = per.tile([1, NVT], f32, name="invnD_row")
        rowtmp = per.tile([1, NVT], f32, name="rowtmp")
        outstage = per.tile([BPT, NTT * DPC], f32, name="outstage")
        dts = [per.tile([128, NVT], proj_dt, name=f"dt{k}") for k in range(KC)]
        qtg = [
            per.tile([128, KC, gw], proj_dt, name=f"qtg{gi}")
            for gi, (_, gw) in enumerate(QGROUPS)
        ]

        # input DMAs: one ring (sync), strict priority order
        nc.sync.dma_start(wt[:], wT_d[:, :, :])
        for k in range(KC):
            nc.sync.dma_start(dts[k][:], dT_d[k * 128:(k + 1) * 128, :])
        for gi, (go, gw) in enumerate(QGROUPS):
            nc.sync.dma_start(qtg[gi][:, :, :], qT_v[:, :, go:go + gw])
        # tiny constants off the critical ring (SWDGE)
        nc.gpsimd.dma_start(qso[:], qso_d[:, :])
        nc.gpsimd.dma_start(onescol[:], onescol_d[:, :])
        nc.gpsimd.dma_start(onesrow[:], onesrow_d[:, :])

        # warm the ACT spline tables while DMAs are in flight
        nc.vector.memset(warm[:], 1.0)
        nc.vector.memset(onescol8[:], 1.0)
        nc.scalar.activation(warm[:], warm[:], AF.Square)
        nc.scalar.activation(warm[:], warm[:], AF.Sqrt)

        # ---------------- phase D: project doc tokens ----------------------
        # k-outer accumulation into one wide PSUM tensor so compute starts
        # as soon as the first dT k-chunk lands.
        with (
            tc.tile_pool(name="psD", bufs=1, space="PSUM") as psD,
            tc.tile_pool(name="ssD", bufs=1, space="PSUM") as ssD,
            tc.tile_pool(name="sqD_pool", bufs=2) as sqD_pool,
            tc.tile_pool(name="bc_pool", bufs=2) as bc_pool,
            tc.tile_pool(name="psB", bufs=2, space="PSUM") as psB,
        ):
            # psd split per 512-chunk so each chunk's PSUM bank frees right
            # after its DTn multiply (the Q-projection PSUM reuses them)
            psds = [
                psD.tile([128, ln], f32, name=f"psd{ci}")
                for ci, (off, ln) in enumerate(d_chunks)
            ]
            for k in range(KC):
                for ci, (off, ln) in enumerate(d_chunks):
                    nc.tensor.matmul(
                        psds[ci][:, :ln],
                        wt[:, k, :],
                        dts[k][:, off:off + ln],
                        start=(k == 0),
                        stop=(k == KC - 1),
                    )
            # chunk-granular norm chain: Square (ACT) -> M=8 ones matmul (PE,
            # full-rate) -> sqrt row straight from PSUM (ACT) -> ~51-ULP
            # reciprocal (DVE); stages pipeline across the 5 chunks
            for ci, (off, ln) in enumerate(d_chunks):
                sl = slice(off, off + ln)
                sq = sqD_pool.tile([128, 512], sq_dt, name="sqd", tag="sq")
                nc.scalar.activation(sq[:, :ln], psds[ci][:, :ln], AF.Square)
                ssd = ssD.tile([8, 512], f32, name="ssd", tag="ssd")
                nc.tensor.matmul(
                    ssd[:, :ln], onescol8[:], sq[:, :ln], start=True, stop=True
                )
                nc.scalar.activation(rowtmp[0:1, sl], ssd[0:1, :ln], AF.Sqrt)
                nc.vector.reciprocal_approx_fast(
                    invnD_row[0:1, sl], rowtmp[0:1, sl]
                )

            # broadcast 1/|D| across partitions and scale D straight out of
            # the projection PSUM (each psd chunk dies at its multiply)
            for ci, (off, ln) in enumerate(d_chunks):
                sl = slice(off, off + ln)
                psb = psB.tile([128, 512], f32, name="psb", tag="psb")
                nc.tensor.matmul(
                    psb[:, :ln], onesrow[:], invnD_row[:, sl], start=True, stop=True
                )
                bc = bc_pool.tile([128, 512], f32, name="bcast_sb", tag="bc")
                if ci < 3:
                    nc.scalar.copy(bc[:, :ln], psb[:, :ln])
                else:
                    nc.vector.tensor_scalar_mul(bc[:, :ln], psb[:, :ln], 1.0)
                nc.vector.tensor_tensor(
                    DTn[:, sl], psds[ci][:, :ln], bc[:, :ln], op=ALU.mult
                )

        # ---------- phase Q+S: project query chunks, sim tiles interleaved --
        # Q-projection chunk j feeds sim tiles t=4j..4j+3; chunks are traced
        # two groups ahead of their sim tiles so the PE never starves the DVE
        # reduce pipeline.  pssim is a single 5-bank tensor whose two halves
        # ping-pong between PE writes and DVE reduces.
        with (
            tc.tile_pool(name="psQS", bufs=2, space="PSUM") as psQS,
            tc.tile_pool(name="ssQ", bufs=1, space="PSUM") as ssQ,
            tc.tile_pool(name="sqQ_pool", bufs=2) as sqQ_pool,
            tc.tile_pool(name="psO", bufs=1, space="PSUM") as psO,
            tc.tile_pool(name="m_pool", bufs=2) as m_pool,
            tc.tile_pool(name="fold_pool", bufs=2) as fold_pool,
        ):
            ssq = ssQ.tile([128, NTT], f32, name="ssq")
            psout = psO.tile([BPT, NTT * DPC], f32, name="psout")

            psq_live = {}

            def project_mm(j, ks):
                gi, r = _qgroup_of(j)
                if j not in psq_live:
                    psq_live[j] = psQS.tile([128, NVH], f32, name="psq", tag="big")
                psq = psq_live[j]
                for k in ks:
                    nc.tensor.matmul(
                        psq[:, 0:512],
                        wt[:, k, :],
                        qtg[gi][:, k, r:r + 512],
                        start=(k == 0),
                        stop=(k == KC - 1),
                    )

            def project(j):
                sl = slice(j * 512, (j + 1) * 512)
                psq = psq_live.pop(j)
                nc.scalar.copy(QT[:, sl], psq[:, 0:512])
                sq = sqQ_pool.tile([128, 512], sq_dt, name="sqq", tag="sqq")
                nc.scalar.activation(sq[:], psq[:, 0:512], AF.Square)
                for s in range(4):
                    col = j * 4 + s
                    nc.tensor.matmul(
                        ssq[:, col:col + 1],
                        sq[:, s * 128:(s + 1) * 128],
                        onescol[:],
                        start=True,
                        stop=True,
                    )
                # per-chunk 1/|Q| and the weighted block-ones lhsT
                csl = slice(j * 4, (j + 1) * 4)
                nc.scalar.activation(normQ[:, csl], ssq[:, csl], AF.Sqrt)
                nc.vector.reciprocal(invnQ[:, csl], normQ[:, csl])
                nc.vector.tensor_tensor(
                    lhsQ[:, csl, :],
                    qso[:].unsqueeze(1).broadcast_to((128, 4, BPT)),
                    invnQ[:, csl].unsqueeze(2).broadcast_to((128, 4, BPT)),
                    op=ALU.mult,
                )

            def simtile(t):
                lq = QT[:, t * 128:(t + 1) * 128]
                mall = m_pool.tile([128, DPC], f32, name="mall", tag="mall")
                fold = not _is_direct(t)
                if fold:
                    sbt = fold_pool.tile([128, NVT], sim_dt, name="sbt", tag="sbt")
                for h in range(2):
                    base = h * NVH
                    ps = psQS.tile([128, NVH], f32, name="pssim", tag="big")
                    for (off, ln) in s_chunks:
                        nc.tensor.matmul(
                            ps[:, off:off + ln],
                            lq,
                            DTn[:, base + off:base + off + ln],
                            start=True,
                            stop=True,
                        )
                    if fold:
                        # evict to SBUF bf16 on ACT; DVE folds below
                        nc.scalar.copy(sbt[:, base:base + NVH], ps[:])
                    else:
                        nc.vector.reduce_max(
                            mall[:, h * (DPC // 2):(h + 1) * (DPC // 2)],
                            ps[:].rearrange("p (g v) -> p g v", v=NV),
                            axis=AX.X,
                        )
                if fold:
                    # two 2x-rate TT-max fold levels, then a short 1x reduce
                    v2, v4 = NV // 2, NV // 4
                    l1 = fold_pool.tile([128, NVT // 2], sim_dt, name="l1", tag="l1")
                    sv = sbt[:].rearrange("p (g v) -> p g v", v=NV)
                    nc.vector.tensor_tensor(
                        l1[:].rearrange("p (g v) -> p g v", v=v2),
                        sv[:, :, 0:v2], sv[:, :, v2:NV], op=ALU.max,
                    )
                    l2 = fold_pool.tile([128, NVT // 4], sim_dt, name="l2", tag="l2")
                    lv = l1[:].rearrange("p (g v) -> p g v", v=v2)
                    nc.vector.tensor_tensor(
                        l2[:].rearrange("p (g v) -> p g v", v=v4),
                        lv[:, :, 0:v4], lv[:, :, v4:v2], op=ALU.max,
                    )
                    nc.vector.reduce_max(
                        mall[:],
                        l2[:].rearrange("p (g v) -> p g v", v=v4),
                        axis=AX.X,
                    )
                nc.tensor.matmul(
                    psout[:, t * DPC:(t + 1) * DPC],
                    lhsQ[:, t, :],
                    mall[:],
                    start=True,
                    stop=True,
                )

            project_mm(0, range(KC))
            project(0)
            project_mm(1, range(KC))
            project(1)
            for j in range(NQCH):
                for ti, t in enumerate(range(j * 4, (j + 1) * 4)):
                    simtile(t)
                    # chunk j+2's projection as one compact block: the psq
                    # tile only holds a PSUM slot for ~half a tile
                    if j + 2 < NQCH and ti == 2:
                        project_mm(j + 2, range(KC))
                        project(j + 2)
            nc.scalar.copy(outstage[:], psout[:])
            nc.sync.dma_start(
                out_d[:, :].rearrange("(t f) c -> f t c", f=BPT),
                outstage[:].rearrange("f (t c) -> f t c", c=DPC),
            )

    nc.compile()
    return nc


def _host_prep(q_hidden, d_hidden, W, d_mask):
    import ml_dtypes

    bf = ml_dtypes.bfloat16
    q = np.ascontiguousarray(np.asarray(q_hidden, dtype=np.float32))
    d = np.ascontiguousarray(np.asarray(d_hidden, dtype=np.float32))
    w = np.ascontiguousarray(np.asarray(W, dtype=np.float32))
    mask = np.asarray(d_mask, dtype=bool)

    nv = mask.sum(axis=1)
    NV = int(-(-max(int(nv.max()), 16) // 8) * 8)
    NV = min(NV, ((LD + 7) // 8) * 8)

    # per-doc gather indices: valid tokens first, padded with the first
    # valid token (duplicates never change a max)
    idx = np.zeros((B, NV), dtype=np.intp)
    for c in range(B):
        v = np.flatnonzero(mask[c])
        row = np.full(NV, v[0], dtype=np.intp)
        row[:min(len(v), NV)] = v[:NV]
        idx[c] = row

    dG = d[np.arange(B)[:, None], idx, :]          # [B, NV, HID]

    qT = np.ascontiguousarray(q.reshape(TQ, HID).T.astype(bf))   # [HID, TQ]
    # W.T rearranged so the [128, KC, DIM] SBUF tile is one contiguous DMA:
    # wTp[p, k, d] = W[d, k*128+p]
    wT = np.ascontiguousarray(
        w.T.reshape(KC, 128, DIM).transpose(1, 0, 2).astype(bf)
    )
    dT_cores = []
    for m in range(NCORES):
        blk = dG[m * DPC:(m + 1) * DPC].reshape(DPC * NV, HID)
        dT_cores.append(np.ascontiguousarray(blk.T.astype(bf)))  # [HID, DPC*NV]

    qso = np.zeros((128, 128 // LQ), dtype=np.float32)
    for p in range(128):
        qso[p, p // LQ] = 1.0
    onescol = np.ones((128, 1), dtype=bf)
    onesrow = np.ones((1, 128), dtype=np.float32)
    return NV, qT, wT, dT_cores, qso, onescol, onesrow


def kernel(q_hidden, d_hidden, W, d_mask):
    from concourse.bass_utils import run_bass_kernel_spmd

    NV, qT, wT, dT_cores, qso, onescol, onesrow = _host_prep(
        q_hidden, d_hidden, W, d_mask
    )
    nc = _build_program(NV)

    in_maps = [
        {
            "qT": qT,
            "dT": dT_cores[m],
            "wT": wT,
            "qso": qso,
            "onescol": onescol,
            "onesrow": onesrow,
        }
        for m in range(NCORES)
    ]
    res = run_bass_kernel_spmd(nc, in_maps, core_ids=list(range(NCORES)))
    out = np.concatenate(
        [res.results[m]["out"] for m in range(NCORES)], axis=1
    )
    return np.ascontiguousarray(out.astype(np.float32))


# revision 19
# speedup vs baseline: 1.1945x; 1.0495x over previous
"""ColBERT intra-batch MaxSim scoring kernel for 8 Trainium2 NeuronCores.

Math (see reference):
  Q = l2norm(q_hidden @ W.T)                       [B, LQ, DIM]
  D = l2norm(d_hidden @ W.T); D masked             [B, LD, DIM]
  sim[b,c,q,k] = Q[b,q]·D[c,k]; masked k -> -inf
  out[b,c] = sum_q max_k sim

Sharding: docs (dim c) are sharded 16-per-core; q_hidden/W replicated.
Each core computes its [B, 16] slice of the score matrix.

Device-side structure:
  * Host pre-transposes activations to [HID, tokens] and converts to bf16
    (halves HBM traffic; verified 9.6e-4 rel err vs 2e-2 budget).
  * The doc mask is folded away on the host: each doc's valid tokens are
    gathered to the front and the tail is padded with copies of the doc's
    first valid token, so the device kernel needs no masking.
  * All input DMAs are issued on the sync queue in priority order (wt,
    dT chunks, then qT column groups).  One HWDGE ring = strict FIFO, so
    dT gets full bandwidth first and the doc pipeline starts ~15us in;
    qT groups trickle in behind at the rate the sim tiles consume them.
  * Q is NOT normalized before the sim matmul: max_k is invariant under a
    positive per-query scale, so 1/|Q| is folded into the block-ones
    lhsT of the final query-sum matmul.
  * D norms: ones-matmul gives sumsq as a [1, NVT] row; sqrt (ACT) +
    2-ULP approximate reciprocal (DVE) give 1/|D| which is broadcast to
    128 partitions with a K=1 ones outer-product matmul and multiplied in.
  * ACT spline tables for Square/Sqrt are warmed at t=0 so the first
    normalize step doesn't eat the ~2.6us table-load latency.
"""

import os

import numpy as np

B, LQ, LD, HID, DIM = 128, 32, 256, 768, 128
NCORES = 8
DPC = B // NCORES          # docs per core
TQ = B * LQ                # total query tokens
KC = HID // 128            # contraction chunks for the projection

SIM_MODE = os.environ.get("KERNEL_SIM_MODE", "bf16")
# fold path: most tiles take the ACT-convert + DVE TT-max-fold route
# (cheaper on DVE, costs ACT).  The first two tiles and every 4th tile
# from t=6 stay on the direct DVE-reduce path so the DVE starts without
# waiting for ACT, and the two engines stay load-balanced (9 direct / 23
# fold ≈ 66us each at measured per-tile costs).
FOLD = os.environ.get("KERNEL_FOLD", "1") == "1"


def _is_direct(t):
    if not FOLD:
        return True
    return t < 2 or (t >= 6 and (t - 6) % 4 == 0)

# qT column groups, in DMA priority order (first groups smaller so the
# first sim tiles can start as early as possible)
QGROUPS = [(0, 512), (512, 512), (1024, 1024), (2048, 1024), (3072, 1024)]


def _chunks(total, step):
    """[(off, len)] cut at `step` boundaries — a matmul's PSUM output must
    stay inside a single 512-float bank, so chunks may never straddle one."""
    return [(o, min(step, total - o)) for o in range(0, total, step)]


def _qgroup_of(j):
    """(group index, column offset within group) for 512-col chunk j."""
    off = j * 512
    for gi, (go, gw) in enumerate(QGROUPS):
        if go <= off < go + gw:
            return gi, off - go
    raise ValueError(j)


def _build_program(NV):
    import concourse.bass as bass  # noqa: F401
    import concourse.tile as tile
    from concourse import bacc, mybir

    f32 = mybir.dt.float32
    bf16 = mybir.dt.bfloat16
    AF = mybir.ActivationFunctionType
    AX = mybir.AxisListType
    ALU = mybir.AluOpType

    proj_dt = bf16
    sim_dt = {"bf16": bf16, "f32": f32}[SIM_MODE]
    sq_dt = bf16

    NVT = DPC * NV          # compacted doc tokens per core
    NVH = NVT // 2          # half (8 docs) — one PSUM sim tile
    NQCH = TQ // 512        # q-projection column chunks
    NTT = TQ // 128         # sim lhsT tiles (query-token tiles)
    BPT = 128 // LQ         # batch entries per query-token tile
    d_chunks = _chunks(NVT, 512)   # d-projection column chunks
    s_chunks = _chunks(NVH, 512)   # sim matmul N chunks per half

    nc = bacc.Bacc(
        "TRN2",
        target_bir_lowering=False,
        debug=False,
        num_devices=NCORES,
    )

    qT_d = nc.dram_tensor("qT", [HID, TQ], proj_dt, kind="ExternalInput")
    dT_d = nc.dram_tensor("dT", [HID, NVT], proj_dt, kind="ExternalInput")
    wT_d = nc.dram_tensor("wT", [128, KC, DIM], proj_dt, kind="ExternalInput")
    qso_d = nc.dram_tensor("qso", [128, BPT], f32, kind="ExternalInput")
    onescol_d = nc.dram_tensor("onescol", [128, 1], sq_dt, kind="ExternalInput")
    onesrow_d = nc.dram_tensor("onesrow", [1, 128], f32, kind="ExternalInput")
    out_d = nc.dram_tensor("out", [B, DPC], f32, kind="ExternalOutput")

    # [HID, t] rows seen as (k, p): row = k*128 + p
    qT_v = qT_d[:, :].rearrange("(k p) t -> p k t", p=128)

    with tile.TileContext(nc) as tc, tc.tile_pool(name="persist", bufs=1) as per:
        # --- constants + persistent SBUF tensors ---------------------------
        wt = per.tile([128, KC, DIM], proj_dt, name="wt")
        qso = per.tile([128, BPT], f32, name="qso")
        onescol = per.tile([128, 1], sq_dt, name="onescol")
        onescol8 = per.tile([128, 8], sq_dt, name="onescol8")
        onesrow = per.tile([1, 128], f32, name="onesrow")
        warm = per.tile([1, 16], f32, name="warm")
        QT = per.tile([128, TQ], sim_dt, name="QT")       # q-proj [d, t] unnormalized
        DTn = per.tile([128, NVT], sim_dt, name="DTn")    # normalized d-proj
        invnQ = per.tile([128, NTT], f32, name="invnQ")   # 1/|Q| per query token
        normQ = per.tile([128, NTT], f32, name="normQ")
        lhsQ = per.tile([128, NTT, BPT], f32, name="lhsQ")  # blockones * 1/|Q|
        invnD_row = per.tile([1, NVT], f32, name="invnD_row")
        rowtmp = per.tile([1, NVT], f32, name="rowtmp")
        outstage = per.tile([BPT, NTT * DPC], f32, name="outstage")
        dts = [per.tile([128, NVT], proj_dt, name=f"dt{k}") for k in range(KC)]
        qtg = [
            per.tile([128, KC, gw], proj_dt, name=f"qtg{gi}")
            for gi, (_, gw) in enumerate(QGROUPS)
        ]

        # input DMAs: one ring (sync), strict priority order
        nc.sync.dma_start(wt[:], wT_d[:, :, :])
        for k in range(KC):
            nc.sync.dma_start(dts[k][:], dT_d[k * 128:(k + 1) * 128, :])
        for gi, (go, gw) in enumerate(QGROUPS):
            nc.sync.dma_start(qtg[gi][:, :, :], qT_v[:, :, go:go + gw])
        # tiny constants off the critical ring (SWDGE)
        nc.gpsimd.dma_start(qso[:], qso_d[:, :])
        nc.gpsimd.dma_start(onescol[:], onescol_d[:, :])
        nc.gpsimd.dma_start(onesrow[:], onesrow_d[:, :])

        # warm the ACT spline tables while DMAs are in flight
        nc.vector.memset(warm[:], 1.0)
        nc.vector.memset(onescol8[:], 1.0)
        nc.scalar.activation(warm[:], warm[:], AF.Square)
        nc.scalar.activation(warm[:], warm[:], AF.Sqrt)

        # ---------------- phase D: project doc tokens ----------------------
        # k-outer accumulation into one wide PSUM tensor so compute starts
        # as soon as the first dT k-chunk lands.
        with (
            tc.tile_pool(name="psD", bufs=1, space="PSUM") as psD,
            tc.tile_pool(name="ssD", bufs=1, space="PSUM") as ssD,
            tc.tile_pool(name="sqD_pool", bufs=2) as sqD_pool,
            tc.tile_pool(name="bc_pool", bufs=2) as bc_pool,
            tc.tile_pool(name="psB", bufs=2, space="PSUM") as psB,
        ):
            # psd split per 512-chunk so each chunk's PSUM bank frees right
            # after its DTn multiply (the Q-projection PSUM reuses them)
            psds = [
                psD.tile([128, ln], f32, name=f"psd{ci}")
                for ci, (off, ln) in enumerate(d_chunks)
            ]
            for k in range(KC):
                for ci, (off, ln) in enumerate(d_chunks):
                    nc.tensor.matmul(
                        psds[ci][:, :ln],
                        wt[:, k, :],
                        dts[k][:, off:off + ln],
                        start=(k == 0),
                        stop=(k == KC - 1),
                    )
            # chunk-granular norm chain: Square (ACT) -> M=8 ones matmul (PE,
            # full-rate) -> sqrt row straight from PSUM (ACT) -> ~51-ULP
            # reciprocal (DVE); stages pipeline across the 5 chunks
            for ci, (off, ln) in enumerate(d_chunks):
                sl = slice(off, off + ln)
                sq = sqD_pool.tile([128, 512], sq_dt, name="sqd", tag="sq")
                nc.scalar.activation(sq[:, :ln], psds[ci][:, :ln], AF.Square)
                ssd = ssD.tile([8, 512], f32, name="ssd", tag="ssd")
                nc.tensor.matmul(
                    ssd[:, :ln], onescol8[:], sq[:, :ln], start=True, stop=True
                )
                nc.scalar.activation(rowtmp[0:1, sl], ssd[0:1, :ln], AF.Sqrt)
                nc.vector.reciprocal_approx_fast(
                    invnD_row[0:1, sl], rowtmp[0:1, sl]
                )

            # broadcast 1/|D| across partitions and scale D straight out of
            # the projection PSUM (each psd chunk dies at its multiply)
            for ci, (off, ln) in enumerate(d_chunks):
                sl = slice(off, off + ln)
                psb = psB.tile([128, 512], f32, name="psb", tag="psb")
                nc.tensor.matmul(
                    psb[:, :ln], onesrow[:], invnD_row[:, sl], start=True, stop=True
                )
                bc = bc_pool.tile([128, 512], f32, name="bcast_sb", tag="bc")
                if ci < 3:
                    nc.scalar.copy(bc[:, :ln], psb[:, :ln])
                else:
                    nc.vector.tensor_scalar_mul(bc[:, :ln], psb[:, :ln], 1.0)
                nc.vector.tensor_tensor(
                    DTn[:, sl], psds[ci][:, :ln], bc[:, :ln], op=ALU.mult
                )

        # ---------- phase Q+S: project query chunks, sim tiles interleaved --
        # Q-projection chunk j feeds sim tiles t=4j..4j+3; chunks are traced
        # two groups ahead of their sim tiles so the PE never starves the DVE
        # reduce pipeline.  pssim is a single 5-bank tensor whose two halves
        # ping-pong between PE writes and DVE reduces.
        with (
            tc.tile_pool(name="psQS", bufs=2, space="PSUM") as psQS,
            tc.tile_pool(name="ssQ", bufs=1, space="PSUM") as ssQ,
            tc.tile_pool(name="sqQ_pool", bufs=2) as sqQ_pool,
            tc.tile_pool(name="psO", bufs=1, space="PSUM") as psO,
            tc.tile_pool(name="m_pool", bufs=6) as m_pool,
            tc.tile_pool(name="fold_pool", bufs=4) as fold_pool,
        ):
            ssq = ssQ.tile([128, NTT], f32, name="ssq")
            psout = psO.tile([BPT, NTT * DPC], f32, name="psout")

            psq_live = {}

            def project_mm(j, ks):
                gi, r = _qgroup_of(j)
                if j not in psq_live:
                    psq_live[j] = psQS.tile([128, NVH], f32, name="psq", tag="big")
                psq = psq_live[j]
                for k in ks:
                    nc.tensor.matmul(
                        psq[:, 0:512],
                        wt[:, k, :],
                        qtg[gi][:, k, r:r + 512],
                        start=(k == 0),
                        stop=(k == KC - 1),
                    )

            def project(j):
                sl = slice(j * 512, (j + 1) * 512)
                psq = psq_live.pop(j)
                nc.scalar.copy(QT[:, sl], psq[:, 0:512])
                sq = sqQ_pool.tile([128, 512], sq_dt, name="sqq", tag="sqq")
                nc.scalar.activation(sq[:], psq[:, 0:512], AF.Square)
                for s in range(4):
                    col = j * 4 + s
                    nc.tensor.matmul(
                        ssq[:, col:col + 1],
                        sq[:, s * 128:(s + 1) * 128],
                        onescol[:],
                        start=True,
                        stop=True,
                    )
                # per-chunk 1/|Q| and the weighted block-ones lhsT
                csl = slice(j * 4, (j + 1) * 4)
                nc.scalar.activation(normQ[:, csl], ssq[:, csl], AF.Sqrt)
                nc.vector.reciprocal(invnQ[:, csl], normQ[:, csl])
                nc.vector.tensor_tensor(
                    lhsQ[:, csl, :],
                    qso[:].unsqueeze(1).broadcast_to((128, 4, BPT)),
                    invnQ[:, csl].unsqueeze(2).broadcast_to((128, 4, BPT)),
                    op=ALU.mult,
                )

            def fold_finish(t, sbt):
                # two 2x-rate TT-max fold levels, then a short 1x reduce;
                # issued a couple of tiles late so the DVE's inputs are
                # always ready (no cross-engine just-in-time stalls)
                mall = m_pool.tile([128, DPC], f32, name="mall", tag="mall")
                v2, v4 = NV // 2, NV // 4
                l1 = fold_pool.tile([128, NVT // 2], sim_dt, name="l1", tag="l1")
                sv = sbt[:].rearrange("p (g v) -> p g v", v=NV)
                nc.vector.tensor_tensor(
                    l1[:].rearrange("p (g v) -> p g v", v=v2),
                    sv[:, :, 0:v2], sv[:, :, v2:NV], op=ALU.max,
                )
                l2 = fold_pool.tile([128, NVT // 4], sim_dt, name="l2", tag="l2")
                lv = l1[:].rearrange("p (g v) -> p g v", v=v2)
                nc.vector.tensor_tensor(
                    l2[:].rearrange("p (g v) -> p g v", v=v4),
                    lv[:, :, 0:v4], lv[:, :, v4:v2], op=ALU.max,
                )
                nc.vector.reduce_max(
                    mall[:],
                    l2[:].rearrange("p (g v) -> p g v", v=v4),
                    axis=AX.X,
                )
                nc.tensor.matmul(
                    psout[:, t * DPC:(t + 1) * DPC],
                    lhsQ[:, t, :],
                    mall[:],
                    start=True,
                    stop=True,
                )

            def simtile(t):
                lq = QT[:, t * 128:(t + 1) * 128]
                fold = not _is_direct(t)
                if fold:
                    sbt = fold_pool.tile([128, NVT], sim_dt, name="sbt", tag="sbt")
                else:
                    mall = m_pool.tile([128, DPC], f32, name="mall", tag="mall")
                for h in range(2):
                    base = h * NVH
                    ps = psQS.tile([128, NVH], f32, name="pssim", tag="big")
                    for (off, ln) in s_chunks:
                        nc.tensor.matmul(
                            ps[:, off:off + ln],
                            lq,
                            DTn[:, base + off:base + off + ln],
                            start=True,
                            stop=True,
                        )
                    if fold:
                        # evict to SBUF bf16 on ACT; DVE folds later
                        nc.scalar.copy(sbt[:, base:base + NVH], ps[:])
                    else:
                        nc.vector.reduce_max(
                            mall[:, h * (DPC // 2):(h + 1) * (DPC // 2)],
                            ps[:].rearrange("p (g v) -> p g v", v=NV),
                            axis=AX.X,
                        )
                if fold:
                    return (t, sbt)
                nc.tensor.matmul(
                    psout[:, t * DPC:(t + 1) * DPC],
                    lhsQ[:, t, :],
                    mall[:],
                    start=True,
                    stop=True,
                )
                return None

            project_mm(0, range(KC))
            project(0)
            project_mm(1, range(KC))
            project(1)
            pending = []
            for j in range(NQCH):
                for ti, t in enumerate(range(j * 4, (j + 1) * 4)):
                    p = simtile(t)
                    if p is not None:
                        pending.append(p)
                    if len(pending) > 2:
                        fold_finish(*pending.pop(0))
                    # chunk j+2's projection as one compact block: the psq
                    # tile only holds a PSUM slot for ~half a tile
                    if j + 2 < NQCH and ti == 2:
                        project_mm(j + 2, range(KC))
                        project(j + 2)
            for p in pending:
                fold_finish(*p)
            nc.scalar.copy(outstage[:], psout[:])
            nc.sync.dma_start(
                out_d[:, :].rearrange("(t f) c -> f t c", f=BPT),
                outstage[:].rearrange("f (t c) -> f t c", c=DPC),
            )

    nc.compile()
    return nc


def _host_prep(q_hidden, d_hidden, W, d_mask):
    import ml_dtypes

    bf = ml_dtypes.bfloat16
    q = np.ascontiguousarray(np.asarray(q_hidden, dtype=np.float32))
    d = np.ascontiguousarray(np.asarray(d_hidden, dtype=np.float32))
    w = np.ascontiguousarray(np.asarray(W, dtype=np.float32))
    mask = np.asarray(d_mask, dtype=bool)

    nv = mask.sum(axis=1)
    NV = int(-(-max(int(nv.max()), 16) // 8) * 8)
    NV = min(NV, ((LD + 7) // 8) * 8)

    # per-doc gather indices: valid tokens first, padded with the first
    # valid token (duplicates never change a max)
    idx = np.zeros((B, NV), dtype=np.intp)
    for c in range(B):
        v = np.flatnonzero(mask[c])
        row = np.full(NV, v[0], dtype=np.intp)
        row[:min(len(v), NV)] = v[:NV]
        idx[c] = row

    dG = d[np.arange(B)[:, None], idx, :]          # [B, NV, HID]

    qT = np.ascontiguousarray(q.reshape(TQ, HID).T.astype(bf))   # [HID, TQ]
    # W.T rearranged so the [128, KC, DIM] SBUF tile is one contiguous DMA:
    # wTp[p, k, d] = W[d, k*128+p]
    wT = np.ascontiguousarray(
        w.T.reshape(KC, 128, DIM).transpose(1, 0, 2).astype(bf)
    )
    dT_cores = []
    for m in range(NCORES):
        blk = dG[m * DPC:(m + 1) * DPC].reshape(DPC * NV, HID)
        dT_cores.append(np.ascontiguousarray(blk.T.astype(bf)))  # [HID, DPC*NV]

    qso = np.zeros((128, 128 // LQ), dtype=np.float32)
    for p in range(128):
        qso[p, p // LQ] = 1.0
    onescol = np.ones((128, 1), dtype=bf)
    onesrow = np.ones((1, 128), dtype=np.float32)
    return NV, qT, wT, dT_cores, qso, onescol, onesrow


def kernel(q_hidden, d_hidden, W, d_mask):
    from concourse.bass_utils import run_bass_kernel_spmd

    NV, qT, wT, dT_cores, qso, onescol, onesrow = _host_prep(
        q_hidden, d_hidden, W, d_mask
    )
    nc = _build_program(NV)

    in_maps = [
        {
            "qT": qT,
            "dT": dT_cores[m],
            "wT": wT,
            "qso": qso,
            "onescol": onescol,
            "onesrow": onesrow,
        }
        for m in range(NCORES)
    ]
    res = run_bass_kernel_spmd(nc, in_maps, core_ids=list(range(NCORES)))
    out = np.concatenate(
        [res.results[m]["out"] for m in range(NCORES)], axis=1
    )
    return np.ascontiguousarray(out.astype(np.float32))
